# revision 50
# baseline (speedup 1.0000x reference)
"""Distributed (8-NeuronCore SPMD) Trainium2 Bass kernel: masked multi-head attention.

Problem: x[4,2048,1024] -> qkv (16 heads, d=64) -> masked softmax attention -> proj.

Sharding (Megatron-style, per sharding hint):
  core c -> batch b = c//2, head-group g = c%2 (8 heads per core).
  Wqkv columns / Wproj rows are split by head group; each core computes a full
  [2048,1024] *partial* projection output for its batch; the host sums the two
  tensor-parallel partials per batch (bproj is fed to group 0 only).

Per-core kernel layout strategy:
  - Q^T,K^T computed weight-stationary -> [channels, tokens] layout in SBUF.
  - V computed x^T-stationary -> natural [tokens, channels] layout, augmented
    with a ones column per head ([V | 1]).
  - Scores are computed transposed: S^T[m,q] = sum_d K^T[d,m] Q^T[d,q] (the
    1/sqrt(d) scale is folded into Wk/bk on the host), masked-exp'd into
    P^T[m,q] (bf16), then O^T[d,q] = sum_m V[m,d] P^T[m,q] accumulates in PSUM.
    The ones column makes row 64 of the PV output the softmax denominator.
  - Normalization: denominators broadcast across partitions via a tiny
    ones-outer-product matmul, fast-reciprocal + multiply on the vector engine.
  - Projection: O^T-stationary, Wproj moving -> natural [tokens, 1024] output,
    staged through SBUF to DRAM.
  - Scheduling: head-pair pipeline; qkv(hp+1) matmuls are emitted after
    attention(hp) so they fill TensorE gaps of the ACT/DVE-paced softmax loop
    and keep the HAM clock-gate at full rate.
"""

import os

import numpy as np
import ml_dtypes

BF16 = ml_dtypes.bfloat16

B, N, DIM, HEADS = 4, 2048, 1024, 16
HL = HEADS // 2        # heads per core = 8
D = DIM // HEADS       # head dim = 64
CL = DIM // 2          # local channels per core = 512
P = 128
NCH = N // P           # 16 token chunks
CCH = DIM // P         # 8 contraction chunks
VW = D + 1             # 65: V plus ones column

_nc_cache = None
LAST_EXEC_NS = None
LAST_RESULTS = None


def _body(tc, nc, mybir, xT, wqk, wv, bqk, bv, maskk, wp, bp, out):
    import concourse.bass as bass  # noqa: F401

    f32 = mybir.dt.float32
    bf16 = mybir.dt.bfloat16
    Exp = mybir.ActivationFunctionType.Exp
    HP = HL // 2  # head pairs per core = 4

    with (
        tc.tile_pool(name="persist", bufs=1) as pers,
        tc.tile_pool(name="qkp", bufs=2) as qkpool,
        tc.tile_pool(name="wqkp", bufs=2) as wqkpool,
        tc.tile_pool(name="pp", bufs=8) as ppool,
        tc.tile_pool(name="sp", bufs=4) as spool,
        tc.tile_pool(name="rp", bufs=3) as rpool,
        tc.tile_pool(name="zp", bufs=2) as zpool,
        tc.tile_pool(name="pb", bufs=2, space="PSUM") as pb,
        tc.tile_pool(name="pc", bufs=2, space="PSUM") as pc,
    ):
        # ---------------- persistent tiles ----------------
        xt = pers.tile([P, CCH, N], bf16, name="xt")          # x^T, [c, n]
        msk = pers.tile([P, NCH, N], mybir.dt.uint8, name="msk")  # keep: 1 = attend
        vsb = pers.tile([P, NCH, HL * VW], bf16, name="vsb")  # V natural, [V | 1] per head
        wpsb = pers.tile([P, CL // P, DIM], bf16, name="wpsb")
        onrm = pers.tile([P, CL // P, N], bf16, name="onrm")  # normalized O^T, [c, n]
        bqksb = pers.tile([P, 2 * CL // P], f32, name="bqksb")
        bvsb = pers.tile([1, CL], bf16, name="bvsb")
        bpsb = pers.tile([1, DIM], bf16, name="bpsb")
        ones = pers.tile([1, P], bf16, name="ones")

        nc.vector.memset(ones[:], 1.0)
        nc.vector.memset(
            vsb.rearrange("p t (h c) -> p t h c", c=VW)[:, :, :, D], 1.0
        )
        # only what qkv(0)/vphase need up front; bulk mask/proj loads are
        # emitted later (lower priority) so compute starts immediately
        for c in range(CCH):
            # split per chunk so the first chunk lands fast (2 DMA queues)
            nc.sync.dma_start(xt[:, c, :1024], xT[c * P:(c + 1) * P, :1024])
            nc.sync.dma_start(xt[:, c, 1024:], xT[c * P:(c + 1) * P, 1024:])
        nc.sync.dma_start(bqksb[:], bqk.rearrange("(o p) -> p o", p=P))
        nc.sync.dma_start(bvsb[:], bv[:])

        def bulk_loads():
            for t in range(NCH):
                nc.sync.dma_start(msk[:, t, :], maskk[t * P:(t + 1) * P, :])
            for o in range(CL // P):
                nc.sync.dma_start(wpsb[:, o, :], wp[o * P:(o + 1) * P, :])
            nc.sync.dma_start(bpsb[:], bp[:])

        # Per-head-pair pipeline: qkv(hp) is emitted before attention(hp);
        # qkv(hp+1) is emitted after it, so its matmuls fill the PE gaps of
        # the ACT-paced attention loop and keep the HAM clock-gate warm.
        def vphase():
            # V for all heads: x^T stationary, Wv moving -> natural layout
            with tc.tile_pool(name="wvp", bufs=1) as wvpool:
                wvsb = wvpool.tile([P, CCH, CL], bf16, name="wvsb")
                for c in range(CCH):
                    nc.sync.dma_start(wvsb[:, c, :], wv[c * P:(c + 1) * P, :])
                for t in range(NCH):
                    vps = pb.tile([P, 1024], f32, name=f"vps{t}", tag="pb")
                    for c in range(CCH):
                        nc.tensor.matmul(
                            vps[:, :CL],
                            xt[:, c, t * P:(t + 1) * P],
                            wvsb[:, c, :],
                            start=(c == 0),
                            stop=False,
                        )
                    nc.tensor.matmul(
                        vps[:, :CL], ones[:, :], bvsb[:, :], start=False, stop=True
                    )
                    nc.vector.tensor_copy(
                        vsb.rearrange("p t (h c) -> p t h c", c=VW)[:, t, :, :D],
                        vps[:, :CL].rearrange("p (h c) -> p h c", c=D),
                    )

        def qkv(hp):
            wq = wqkpool.tile([P, CCH, 2 * P], bf16, name=f"wq{hp}", tag="wqk")
            for c in range(CCH):
                nc.sync.dma_start(
                    wq[:, c, :P], wqk[c * P:(c + 1) * P, P * hp:P * (hp + 1)]
                )
                nc.sync.dma_start(
                    wq[:, c, P:], wqk[c * P:(c + 1) * P, CL + P * hp:CL + P * (hp + 1)]
                )
            # Q^T (j01=0) and K^T (j01=1): weights stationary, x^T moving
            qk = qkpool.tile([P, 2, N], bf16, name=f"qk{hp}", tag="qk")
            for j01 in range(2):
                bcol = hp if j01 == 0 else CCH // 2 + hp
                # one PSUM slot at a time so gap-filler matmuls never starve
                # the attention score tiles of pb slots
                for nh in range(2):
                    ps = pb.tile([P, 1024], f32, name=f"qs{hp}_{j01}_{nh}", tag="pb")
                    for c in range(CCH):
                        lhs = wq[:, c, j01 * P:(j01 + 1) * P]
                        for s in range(2):
                            nc.tensor.matmul(
                                ps[:, s * 512:(s + 1) * 512],
                                lhs,
                                xt[:, c, nh * 1024 + s * 512:nh * 1024 + (s + 1) * 512],
                                start=(c == 0),
                                stop=(c == CCH - 1),
                            )
                    nc.vector.tensor_scalar_add(
                        qk[:, j01, nh * 1024:(nh + 1) * 1024],
                        ps[:],
                        bqksb[:, bcol:bcol + 1],
                    )
            return qk

        def attention(hp, qk, us=(0, 1)):
            for u in us:  # q-half
                qlo = u * 1024
                otp = [
                    pc.tile([VW, 1024], f32, name=f"ot{hp}_{u}_{e}", tag="pc")
                    for e in range(2)
                ]
                for mi in range(NCH):
                    st = [
                        pb.tile([P, 1024], f32, name=f"s{hp}_{u}_{mi}_{e}", tag="pb")
                        for e in range(2)
                    ]
                    for s in range(2):
                        for e in range(2):  # pair member -> PE row group
                            row = D * e
                            nc.tensor.matmul(
                                st[e][:, s * 512:(s + 1) * 512],
                                qk[row:row + D, 1, mi * P:(mi + 1) * P],
                                qk[row:row + D, 0, qlo + s * 512:qlo + (s + 1) * 512],
                                start=True,
                                stop=True,
                            )
                    # one [128, 2, 1024] P tile: exp on ACT, then one fused
                    # keep-mask multiply for both heads (u8 mask broadcast);
                    # alternate DVE / GpSimd so neither engine paces the loop
                    pt = ppool.tile([P, 2, 1024], bf16, name=f"p{hp}_{u}_{mi}", tag="pp")
                    for e in range(2):
                        nc.scalar.activation(pt[:, e, :], st[e][:], Exp)
                    meng = nc.vector
                    meng.tensor_tensor(
                        pt[:, :, :],
                        pt[:, :, :],
                        msk[:, mi, None, qlo:qlo + 1024].to_broadcast([P, 2, 1024]),
                        mybir.AluOpType.mult,
                    )
                    for e in range(2):
                        h = 2 * hp + e
                        for s in range(2):
                            nc.tensor.matmul(
                                otp[e][:, s * 512:(s + 1) * 512],
                                vsb[:, mi, VW * h:VW * h + VW],
                                pt[:, e, s * 512:(s + 1) * 512],
                                start=(mi == 0),
                                stop=(mi == NCH - 1),
                            )
                # normalize: row D of otp[e] is the softmax denominator.
                # Stage otp out through one fast ACT copy so the PSUM slots
                # free immediately for the next unit's PV accumulators.
                for e in range(2):
                    row = D * e
                    ost = zpool.tile([VW, 1024], f32, name=f"ost{hp}_{e}_{u}", tag="zs")
                    nc.scalar.copy(ost[:], otp[e][:])
                    ssb = spool.tile([1, 1024], bf16, name=f"ssb{hp}_{e}_{u}", tag="sp")
                    nc.vector.tensor_copy(ssb[:], ost[D:D + 1, :])
                    sbc = pc.tile([D, 1024], f32, name=f"sbc{hp}_{e}_{u}", tag="pc")
                    for s in range(2):
                        nc.tensor.matmul(
                            sbc[:, s * 512:(s + 1) * 512],
                            ones[:, :D],
                            ssb[:, s * 512:(s + 1) * 512],
                            start=True,
                            stop=True,
                        )
                    rb = rpool.tile([D, 1024], f32, name=f"rb{hp}_{e}_{u}", tag="rp")
                    nc.vector.reciprocal_approx_fast(rb[:], sbc[:])
                    nc.vector.tensor_mul(
                        onrm[row:row + D, hp, qlo:qlo + 1024],
                        ost[:D, :],
                        rb[:],
                    )

        def proj(t):
            zp = pb.tile([P, 1024], f32, name=f"z{t}", tag="pb")
            for s in range(2):
                for c in range(CL // P):
                    nc.tensor.matmul(
                        zp[:, s * 512:(s + 1) * 512],
                        onrm[:, c, t * P:(t + 1) * P],
                        wpsb[:, c, s * 512:(s + 1) * 512],
                        start=(c == 0),
                        stop=False,
                    )
                nc.tensor.matmul(
                    zp[:, s * 512:(s + 1) * 512],
                    ones[:, :],
                    bpsb[:, s * 512:(s + 1) * 512],
                    start=False,
                    stop=True,
                )
            zs = zpool.tile([P, 1024], f32, name=f"zs{t}", tag="zs")
            nc.scalar.copy(zs[:], zp[:])
            nc.sync.dma_start(out[t * P:(t + 1) * P, :], zs[:])

        prev = qkv(0)
        vphase()
        bulk_loads()
        for hp in range(HP):
            attention(hp, prev)
            # emitted after attention(hp) => lower priority => its matmuls
            # fill the PE idle slots of the ACT-paced attention loop
            prev = qkv(hp + 1) if hp + 1 < HP else None
        for t in range(NCH):
            proj(t)


def _build_nc():
    import concourse.tile as tile
    from concourse import bacc, mybir

    f32 = mybir.dt.float32
    bf16 = mybir.dt.bfloat16

    nc = bacc.Bacc("TRN2", target_bir_lowering=False, debug=False)

    xT = nc.declare_dram_parameter("xT", [DIM, N], bf16, isOutput=False)
    wqk = nc.declare_dram_parameter("wqk", [DIM, 2 * CL], bf16, isOutput=False)
    wv = nc.declare_dram_parameter("wv", [DIM, CL], bf16, isOutput=False)
    bqk = nc.declare_dram_parameter("bqk", [2 * CL], f32, isOutput=False)
    bv = nc.declare_dram_parameter("bv", [1, CL], bf16, isOutput=False)
    maskk = nc.declare_dram_parameter("maskk", [N, N], mybir.dt.uint8, isOutput=False)
    wp = nc.declare_dram_parameter("wp", [CL, DIM], bf16, isOutput=False)
    bp = nc.declare_dram_parameter("bp", [1, DIM], bf16, isOutput=False)
    out = nc.declare_dram_parameter("out", [N, DIM], f32, isOutput=True)

    with tile.TileContext(nc) as tc:
        _body(tc, nc, mybir, xT, wqk, wv, bqk, bv, maskk, wp, bp, out)
    nc.compile()
    return nc


def _get_nc():
    global _nc_cache
    if _nc_cache is None:
        _nc_cache = _build_nc()
    return _nc_cache


def _shard_inputs(x, mask, Wqkv, bqkv, Wproj, bproj):
    x = np.asarray(x, np.float32)
    mask = np.asarray(mask)
    Wqkv = np.asarray(Wqkv, np.float32)
    bqkv = np.asarray(bqkv, np.float32)
    Wproj = np.asarray(Wproj, np.float32)
    bproj = np.asarray(bproj, np.float32)

    in_maps = []
    for core in range(8):
        b, g = divmod(core, 2)
        qs = slice(CL * g, CL * (g + 1))
        ks = slice(DIM + CL * g, DIM + CL * (g + 1))
        vs = slice(2 * DIM + CL * g, 2 * DIM + CL * (g + 1))
        # softmax 1/sqrt(D) folded into the K weights/bias
        wqk_np = np.concatenate([Wqkv[:, qs], Wqkv[:, ks] * 0.125], axis=1)
        bqk_np = np.concatenate([bqkv[qs], bqkv[ks] * 0.125])
        in_maps.append({
            "xT": np.ascontiguousarray(x[b].T).astype(BF16),
            "wqk": wqk_np.astype(BF16),
            "wv": np.ascontiguousarray(Wqkv[:, vs]).astype(BF16),
            "bqk": bqk_np.astype(np.float32),
            "bv": bqkv[vs].astype(BF16)[None, :],
            # [m, q] layout keep-mask; 1 = attend, 0 = masked (multiplied in)
            "maskk": np.ascontiguousarray(~mask[b].T).astype(np.uint8),
            "wp": np.ascontiguousarray(Wproj[CL * g:CL * (g + 1), :]).astype(BF16),
            "bp": (bproj if g == 0 else np.zeros_like(bproj)).astype(BF16)[None, :],
        })
    return in_maps


def _ensure_ntff_hook():
    """Inject an ``antenv.axon_hooks`` shim (absent on this image) and register
    the ctypes NTFF-profile hook against the loaded libaxon_pjrt.so, so
    ``run_bass_kernel_spmd(trace=True)`` can capture exec_time_ns."""
    import sys
    import types
    import ctypes
    import contextlib

    if "antenv.axon_hooks" not in sys.modules:
        mod = types.ModuleType("antenv.axon_hooks")
        mod._hook = None
        mod.set_axon_ntff_profile_hook = lambda h: setattr(mod, "_hook", h)
        mod.get_axon_ntff_profile_hook = lambda: mod._hook
        sys.modules["antenv.axon_hooks"] = mod
        import antenv

        antenv.axon_hooks = mod

    import antenv.axon_hooks as ah

    if ah.get_axon_ntff_profile_hook() is not None:
        return

    so_path = "/opt/axon/libaxon_pjrt.so"
    if not os.path.exists(so_path):
        return
    lib = ctypes.CDLL(so_path)
    if not hasattr(lib, "axon_start_nrt_profile"):
        return
    lib.axon_start_nrt_profile.argtypes = [
        ctypes.POINTER(ctypes.c_int64),
        ctypes.c_size_t,
    ]
    lib.axon_start_nrt_profile.restype = ctypes.c_int64
    lib.axon_stop_nrt_profile.argtypes = [ctypes.c_char_p]
    lib.axon_stop_nrt_profile.restype = ctypes.c_int64

    @contextlib.contextmanager
    def _hook(output_dir, device_ids):
        import jax

        jax.devices()
        if device_ids:
            ids = (ctypes.c_int64 * len(device_ids))(*device_ids)
            rc = lib.axon_start_nrt_profile(ids, len(device_ids))
        else:
            rc = lib.axon_start_nrt_profile(None, 0)
        if rc != 0:
            raise RuntimeError(f"axon_start_nrt_profile rc={rc}")
        try:
            yield
        finally:
            n = lib.axon_stop_nrt_profile(str(output_dir).encode())
            print(f"ntff profile: {n} file(s) written to {output_dir}")

    ah.set_axon_ntff_profile_hook(_hook)


def kernel(x, mask, Wqkv, bqkv, Wproj, bproj):
    global LAST_EXEC_NS, LAST_RESULTS
    from concourse.bass_utils import run_bass_kernel_spmd

    nc = _get_nc()
    in_maps = _shard_inputs(x, mask, Wqkv, bqkv, Wproj, bproj)
    profile = os.environ.get("BASS_ATTN_PROFILE", "0") == "1"
    if profile:
        _ensure_ntff_hook()
    res = run_bass_kernel_spmd(
        nc, in_maps, core_ids=list(range(8)), trace=profile
    )
    LAST_EXEC_NS = res.exec_time_ns
    LAST_RESULTS = res
    outs = [np.asarray(res.results[c]["out"], np.float32) for c in range(8)]
    return np.stack([outs[2 * b] + outs[2 * b + 1] for b in range(B)], axis=0)
